# revision 7
# baseline (speedup 1.0000x reference)
"""DAF router kernel for 8 trn2 NeuronCores.

Self-contained: hardcodes shapes from the problem spec.
  h [16384, 4096] f32, metadata [16384, 2] f32, k=8,
  w1 [2,16], b1 [16], w2 [16,8], b2 [8], wg [4104, 64], bg [64], mu [64].
Returns (gating_weights [16384,64] f32, selected_indices [16384,8] i32, mu).

Sharding: token axis split across 8 cores (2048 tokens each); router params
replicated. Host pre-transposes h (and metadata) so the device DMAs land
with the contraction dim (D) on SBUF partitions.
"""

import functools
import os

import numpy as np

N, D, E, KTOP = 16384, 4096, 64, 8
M_IN, M_H, M_OUT = 2, 16, 8
NCORES = 8
T = N // NCORES          # 2048 tokens per core
P = 128
C = D // P               # 32 contraction chunks
NG = T // 512            # 4 token groups of 512
JT = T // P              # 16 token tiles of 128
DMA_CHUNKS = 4           # h chunks per DMA (4MB f32)

MODE = os.environ.get("DAF_MODE", "f32r")  # f32 | f32r | bf16x3


def _build(mode: str, reps: int):
    import concourse.bass as bass  # noqa: F401
    import concourse.mybir as mybir
    import concourse.tile as tile
    from concourse import bacc
    from concourse.masks import make_identity

    dt = mybir.dt
    AF = mybir.ActivationFunctionType
    ALU = mybir.AluOpType

    h_dt = {"f32": dt.float32, "f32r": dt.float32r, "bf16x3": dt.bfloat16}[mode]
    wg2_dt = {"f32": dt.float32, "f32r": dt.float32r, "bf16x3": dt.bfloat16}[mode]

    nc = bacc.Bacc("TRN2", target_bir_lowering=False, debug=False, num_devices=NCORES)

    # --- DRAM I/O ---
    if mode == "bf16x3":
        hT_hi = nc.dram_tensor("hT_hi", [D, T], dt.bfloat16, kind="ExternalInput")
        hT_lo = nc.dram_tensor("hT_lo", [D, T], dt.bfloat16, kind="ExternalInput")
        wgh_hi = nc.dram_tensor("wgh_hi", [P, C * E], dt.bfloat16, kind="ExternalInput")
        wgh_lo = nc.dram_tensor("wgh_lo", [P, C * E], dt.bfloat16, kind="ExternalInput")
    else:
        hT = nc.dram_tensor("hT", [D, T], h_dt, kind="ExternalInput")
        wgh = nc.dram_tensor("wgh", [P, C * E], h_dt, kind="ExternalInput")
    mdT = nc.dram_tensor("mdT", [M_IN, T], dt.float32, kind="ExternalInput")
    w1 = nc.dram_tensor("w1", [M_IN, M_H], dt.float32, kind="ExternalInput")
    b1 = nc.dram_tensor("b1", [M_H, 1], dt.float32, kind="ExternalInput")
    w2 = nc.dram_tensor("w2", [M_H, M_OUT], dt.float32, kind="ExternalInput")
    b2 = nc.dram_tensor("b2", [M_OUT, 1], dt.float32, kind="ExternalInput")
    wg2 = nc.dram_tensor("wg2", [M_OUT, E], wg2_dt, kind="ExternalInput")
    bg = nc.dram_tensor("bg", [E, 1], dt.float32, kind="ExternalInput")
    gates = nc.dram_tensor("gates", [T, E], dt.float32, kind="ExternalOutput")
    sidx = nc.dram_tensor("sidx", [T, KTOP], dt.uint32, kind="ExternalOutput")

    with tile.TileContext(nc) as tc:
        with (
            tc.tile_pool(name="const", bufs=1) as const_pool,
            tc.tile_pool(name="hbuf", bufs=2) as h_pool,
            tc.tile_pool(name="work", bufs=2) as work_pool,
            tc.tile_pool(name="tok", bufs=3) as tok_pool,
            tc.tile_pool(name="outbuf", bufs=1) as out_pool,
            tc.tile_pool(name="lg_ps", bufs=1, space="PSUM") as lg_psum,
            tc.tile_pool(name="mlp_ps", bufs=2, space="PSUM") as mlp_psum,
            tc.tile_pool(name="tr_ps", bufs=2, space="PSUM") as tr_psum,
        ):
            # --- persistent constants / weights ---
            ident = const_pool.tile([P, P], dt.float32)
            make_identity(nc, ident[:])

            if mode == "bf16x3":
                wgh_hi_sb = const_pool.tile([P, C, E], dt.bfloat16)
                wgh_lo_sb = const_pool.tile([P, C, E], dt.bfloat16)
                nc.sync.dma_start(wgh_hi_sb[:], wgh_hi.rearrange("p (c e) -> p c e", c=C))
                nc.sync.dma_start(wgh_lo_sb[:], wgh_lo.rearrange("p (c e) -> p c e", c=C))
            else:
                wgh_sb = const_pool.tile([P, C, E], h_dt)
                nc.sync.dma_start(wgh_sb[:], wgh.rearrange("p (c e) -> p c e", c=C))
            mdT_sb = const_pool.tile([M_IN, T], dt.float32)
            nc.sync.dma_start(mdT_sb[:], mdT[:])
            w1_sb = const_pool.tile([M_IN, M_H], dt.float32)
            nc.sync.dma_start(w1_sb[:], w1[:])
            b1_sb = const_pool.tile([M_H, 1], dt.float32)
            nc.sync.dma_start(b1_sb[:], b1[:])
            w2_sb = const_pool.tile([M_H, M_OUT], dt.float32)
            nc.sync.dma_start(w2_sb[:], w2[:])
            b2_sb = const_pool.tile([M_OUT, 1], dt.float32)
            nc.sync.dma_start(b2_sb[:], b2[:])
            wg2_sb = const_pool.tile([M_OUT, E], wg2_dt)
            nc.sync.dma_start(wg2_sb[:], wg2[:])
            bg_sb = const_pool.tile([E, 1], dt.float32)
            nc.sync.dma_start(bg_sb[:], bg[:])

            def body():
                # metadata MLP per token group: m_embT [8, 512] per group
                m_embT_tiles = []
                for g in range(NG):
                    z1_ps = mlp_psum.tile([M_H, 512], dt.float32, tag="mlp")
                    nc.tensor.matmul(
                        z1_ps[:], w1_sb[:], mdT_sb[:, g * 512:(g + 1) * 512],
                        start=True, stop=True,
                    )
                    a1T = work_pool.tile([M_H, 512], dt.float32, tag="a1T")
                    nc.scalar.activation(a1T[:], z1_ps[:], AF.Gelu, bias=b1_sb[:])
                    me_ps = mlp_psum.tile([M_OUT, 512], dt.float32, tag="mlp")
                    nc.tensor.matmul(me_ps[:], w2_sb[:], a1T[:], start=True, stop=True)
                    if mode == "bf16x3":
                        m_embT = work_pool.tile([M_OUT, 512], dt.bfloat16, tag="membT")
                    elif mode == "f32r":
                        m_embT = work_pool.tile([M_OUT, 512], dt.float32r, tag="membT")
                    else:
                        m_embT = work_pool.tile([M_OUT, 512], dt.float32, tag="membT")
                    nc.scalar.activation(m_embT[:], me_ps[:], AF.Identity, bias=b2_sb[:])
                    m_embT_tiles.append(m_embT)

                # logitsT accumulation: [64, 4*512] PSUM
                lg_ps = lg_psum.tile([E, NG, 512], dt.float32)
                for cg in range(C // DMA_CHUNKS):
                    if mode == "bf16x3":
                        hhi_t = h_pool.tile([P, DMA_CHUNKS, T], dt.bfloat16, tag="hhi")
                        hlo_t = h_pool.tile([P, DMA_CHUNKS, T], dt.bfloat16, tag="hlo")
                        src = hT_hi.rearrange("(c p) t -> p c t", p=P)
                        nc.sync.dma_start(
                            hhi_t[:], src[:, cg * DMA_CHUNKS:(cg + 1) * DMA_CHUNKS, :])
                        src = hT_lo.rearrange("(c p) t -> p c t", p=P)
                        nc.sync.dma_start(
                            hlo_t[:], src[:, cg * DMA_CHUNKS:(cg + 1) * DMA_CHUNKS, :])
                    else:
                        h_t = h_pool.tile([P, DMA_CHUNKS, T], h_dt, tag="h")
                        src = hT.rearrange("(c p) t -> p c t", p=P)
                        nc.sync.dma_start(
                            h_t[:], src[:, cg * DMA_CHUNKS:(cg + 1) * DMA_CHUNKS, :])
                    for c4 in range(DMA_CHUNKS):
                        c = cg * DMA_CHUNKS + c4
                        first = c == 0
                        for g in range(NG):
                            if mode == "bf16x3":
                                # (hi*Whi + lo*Whi + hi*Wlo); same stationary
                                # reused for consecutive matmuls.
                                nc.tensor.matmul(
                                    lg_ps[:, g, :], wgh_hi_sb[:, c, :],
                                    hhi_t[:, c4, g * 512:(g + 1) * 512],
                                    start=first, stop=False)
                                nc.tensor.matmul(
                                    lg_ps[:, g, :], wgh_hi_sb[:, c, :],
                                    hlo_t[:, c4, g * 512:(g + 1) * 512],
                                    start=False, stop=False)
                                nc.tensor.matmul(
                                    lg_ps[:, g, :], wgh_lo_sb[:, c, :],
                                    hhi_t[:, c4, g * 512:(g + 1) * 512],
                                    start=False, stop=False)
                            else:
                                nc.tensor.matmul(
                                    lg_ps[:, g, :], wgh_sb[:, c, :],
                                    h_t[:, c4, g * 512:(g + 1) * 512],
                                    start=first, stop=False)

                # output accumulators
                gates_all = out_pool.tile([P, JT, E], dt.float32)
                sidx_all = out_pool.tile([P, JT, KTOP], dt.uint32)

                for g in range(NG):
                    # metadata contribution closes the accumulation group
                    nc.tensor.matmul(lg_ps[:, g, :], wg2_sb[:], m_embT_tiles[g][:],
                                     start=False, stop=True)
                    logitsT = work_pool.tile([E, 512], dt.float32, tag="logitsT")
                    nc.scalar.activation(logitsT[:], lg_ps[:, g, :], AF.Identity,
                                         bias=bg_sb[:])
                    for j4 in range(4):
                        j = g * 4 + j4
                        tr = tr_psum.tile([P, E], dt.float32, tag="tr")
                        nc.tensor.transpose(
                            tr[:], logitsT[:, j4 * P:(j4 + 1) * P], ident[:E, :E])
                        ltok = tok_pool.tile([P, E], dt.float32, tag="ltok")
                        nc.scalar.copy(ltok[:], tr[:])

                        vals8 = tok_pool.tile([P, KTOP], dt.float32, tag="vals8")
                        nc.vector.max(vals8[:], ltok[:])
                        nc.vector.max_index(sidx_all[:, j], vals8[:], ltok[:])

                        negv0 = tok_pool.tile([P, 1], dt.float32, tag="negv0")
                        nc.vector.tensor_scalar_mul(negv0[:], vals8[:, 0:1], -1.0)
                        e8 = tok_pool.tile([P, KTOP], dt.float32, tag="e8")
                        ssum = tok_pool.tile([P, 1], dt.float32, tag="ssum")
                        nc.scalar.activation(e8[:], vals8[:], AF.Exp,
                                             bias=negv0[:], accum_out=ssum[:])
                        efull = tok_pool.tile([P, E], dt.float32, tag="efull")
                        nc.scalar.activation(efull[:], ltok[:], AF.Exp, bias=negv0[:])
                        rinv = tok_pool.tile([P, 1], dt.float32, tag="rinv")
                        nc.vector.reciprocal(rinv[:], ssum[:])
                        # (ltok >= v8) * rinv, then * efull
                        msc = tok_pool.tile([P, E], dt.float32, tag="msc")
                        nc.vector.tensor_scalar(
                            msc[:], ltok[:], vals8[:, KTOP - 1:KTOP], rinv[:],
                            op0=ALU.is_ge, op1=ALU.mult)
                        nc.vector.tensor_tensor(
                            gates_all[:, j], efull[:], msc[:], ALU.mult)

                nc.sync.dma_start(
                    gates.rearrange("(j p) e -> p j e", p=P), gates_all[:])
                nc.sync.dma_start(
                    sidx.rearrange("(j p) e -> p j e", p=P), sidx_all[:])

            if reps == 1:
                body()
            else:
                with tc.For_i(0, reps, 1):
                    body()

    nc.compile()
    return nc


@functools.lru_cache(maxsize=4)
def _get_nc(mode: str, reps: int):
    return _build(mode, reps)


def _split_bf16(x: np.ndarray):
    import ml_dtypes

    hi = x.astype(ml_dtypes.bfloat16)
    lo = (x - hi.astype(np.float32)).astype(ml_dtypes.bfloat16)
    return hi, lo


def _prep_inputs(h, metadata, w1, b1, w2, b2, wg, bg, mode: str):
    h = np.asarray(h, np.float32)
    metadata = np.asarray(metadata, np.float32)
    wg = np.asarray(wg, np.float32)
    # wg rows for h, pre-swizzled to [128, C*E] (chunk-major per partition)
    wgh_m = np.ascontiguousarray(
        wg[:D].reshape(C, P, E).transpose(1, 0, 2)).reshape(P, C * E)
    wg2_m = np.ascontiguousarray(wg[D:])
    common = {
        "w1": np.ascontiguousarray(np.asarray(w1, np.float32)),
        "b1": np.asarray(b1, np.float32).reshape(M_H, 1).copy(),
        "w2": np.ascontiguousarray(np.asarray(w2, np.float32)),
        "b2": np.asarray(b2, np.float32).reshape(M_OUT, 1).copy(),
        "bg": np.asarray(bg, np.float32).reshape(E, 1).copy(),
    }
    if mode == "bf16x3":
        wgh_hi, wgh_lo = _split_bf16(wgh_m)
        common["wgh_hi"], common["wgh_lo"] = wgh_hi, wgh_lo
        common["wg2"] = wg2_m.astype(_split_bf16(wg2_m)[0].dtype)
    else:
        common["wgh"] = wgh_m
        common["wg2"] = wg2_m

    hT = h.T  # [D, N] view
    mdT = metadata.T  # [2, N] view
    in_maps = []
    for core in range(NCORES):
        sl = slice(core * T, (core + 1) * T)
        m = dict(common)
        if mode == "bf16x3":
            hi, lo = _split_bf16(np.ascontiguousarray(hT[:, sl]))
            m["hT_hi"], m["hT_lo"] = hi, lo
        else:
            m["hT"] = np.ascontiguousarray(hT[:, sl])
        m["mdT"] = np.ascontiguousarray(mdT[:, sl])
        in_maps.append(m)
    return in_maps


def run_on_device(h, metadata, w1, b1, w2, b2, wg, bg, mode: str = MODE,
                  reps: int = 1):
    """Run the bass kernel; returns (gates [N,E] f32, sidx [N,8] i32)."""
    from concourse.bass_utils import run_bass_kernel_spmd

    nc = _get_nc(mode, reps)
    in_maps = _prep_inputs(h, metadata, w1, b1, w2, b2, wg, bg, mode)
    res = run_bass_kernel_spmd(nc, in_maps, core_ids=list(range(NCORES)))
    gates = np.concatenate([res.results[i]["gates"] for i in range(NCORES)], axis=0)
    si = np.concatenate([res.results[i]["sidx"] for i in range(NCORES)], axis=0)
    return gates, si.astype(np.int32)


def kernel(h, metadata, k, w1, b1, w2, b2, wg, bg, mu):
    assert int(k) == KTOP
    gates, si = run_on_device(h, metadata, w1, b1, w2, b2, wg, bg, MODE, 1)
    return gates, si, np.asarray(mu, np.float32)
